# revision 35
# baseline (speedup 1.0000x reference)
"""Trainium2 Bass kernel for nn_Net_83794811945603 (3-layer GraphSAGE, mean agg).

Computation (N=50000 nodes, E=800000 edges):
    h0 = x @ W_map + b_map                                  [N,128]
    h1 = relu(mean_agg(h0) @ Wl1 + bl1 + h0 @ Wr1)          [N,128]
    h2 = relu(mean_agg(h1) @ Wl2 + bl2 + h1 @ Wr2)          [N,256]
    out = log_softmax(mean_agg(h2) @ Wl3 + bl3 + h2 @ Wr3)  [N,40]
where mean_agg(h)[i] = mean over edges (s->i) of h[s].

Strategy (8 NeuronCores, SPMD), v2:
  - Nodes sharded row-wise: core c owns nodes [c*6250, (c+1)*6250) and all
    edges whose dst lands there.  Weights replicated.
  - Per layer the full previous-layer feature table lives in DRAM, built by
    a 2-chunk AllGather.  Edge-gather of h[src] rows via SWDGE dma_gather
    (int16 indices over a 25000-row half-table).
  - v2 pipeline: one dma_gather per (half, tile), queue round-robin over
    the 4 SWDGE queues; layers alternate between two gather pools so layer
    L+1's first gathers (which only need AllGather chunk 0 of layer L's
    output) start while layer L is still finishing.
  - Per-(half, tile) gather calls sized by the true max edge count over
    cores (data-dependent static schedule); trailing pad slots carry idx=-1
    which the Q7 ucode trims, so drained bytes track the per-core true
    edge count.
  - Layers run half-sequential: all half-0 tiles aggregate into PSUM and
    park in an SBUF accumulator, then the half-1 pass finishes each tile
    (mean, dense matmuls, output) -- so half-0 gathers overlap the
    second AllGather chunk, and the layer's own output AllGather chunk 0
    fires when tile 24 is written, overlapping the rest of the layer.
  - Segment-sum by dst via TensorE one-hot matmuls: aggT += g_chunk^T @ S,
    S[e, j] = (dst_local[e] == j) built on VectorE (is_equal vs iota).
  - Layer 3 aggregates z3 = h2 @ Wl3 (padded to 128 wide) instead of h2
    (256 wide): segment_mean commutes with the right-matmul.
  - All dense matmuls run transposed (hT = [feat, node]); node-major copies
    for DRAM tables produced with PE transposes.
"""

import os
import sys

sys.path.insert(0, "/opt/trn_rl_repo")

import numpy as np
import ml_dtypes

import concourse.bass as bass
import concourse.bacc as bacc
import concourse.tile as tile
import concourse.mybir as mybir
from concourse.bass_utils import run_bass_kernel_spmd

F32 = mybir.dt.float32
BF16 = mybir.dt.bfloat16
I16 = mybir.dt.int16

N = 50000
E = 800000
F_IN = 500
F_IN_PAD = 512
D = 128            # ID_DIM == HID
D2 = 256           # 2*HID
NCLS = 40
NCLS_PAD = 128
NCORES = 8
NLOC = N // NCORES          # 6250
NT = (NLOC + 127) // 128    # 49 dst tiles per core
NLOC_PAD = NT * 128         # 6272
HALF = N // 2               # int16 index split
NQ = 4                      # SWDGE queues


def _ts(i, size=128):
    return slice(i * size, (i + 1) * size)


def build_program(nch, build_stage=None):
    """Build the SPMD Bass program.

    nch: [2, NT] int array -- chunks (of 128 edge slots) per (half, tile),
    the max over cores of ceil(edge_count/128)."""
    if build_stage is None:
        build_stage = int(os.environ.get("KERNEL_BUILD_STAGE", "3"))
    nc = bacc.Bacc("TRN2", target_bir_lowering=False, debug=False,
                   num_devices=NCORES, num_swdge_queues=NQ)

    nch = np.asarray(nch)
    # segment s = h*NT + t; offsets in dl cols (= global chunk index)
    nch_flat = nch.reshape(-1)
    dl_off = np.concatenate([[0], np.cumsum(nch_flat)])
    DL_COLS = int(dl_off[-1])
    IX_COLS = DL_COLS * 8
    # per-half chunk layout: chunks within a half are contiguous across
    # tiles; gather "pieces" of 8 chunks (1024 idxs, the single-packet cap)
    # are cut across tile boundaries
    HCH = [int(nch[h].sum()) for h in (0, 1)]      # chunks per half
    hch_off = [0, HCH[0]]                          # global chunk offset of half
    NPIECE = [(HCH[h] + 7) // 8 for h in (0, 1)]

    # ---- I/O ----
    xT = nc.dram_tensor("xT", [F_IN_PAD, NLOC_PAD], BF16, kind="ExternalInput")
    idx_d = nc.dram_tensor("idx", [128, IX_COLS], I16, kind="ExternalInput")
    dl_d = nc.dram_tensor("dl", [128, DL_COLS], F32, kind="ExternalInput")
    invdeg_d = nc.dram_tensor("invdeg", [128, NLOC_PAD], F32, kind="ExternalInput")
    iota_d = nc.dram_tensor("iota", [128, 128], F32, kind="ExternalInput")
    ident_d = nc.dram_tensor("ident", [128, 128], BF16, kind="ExternalInput")
    ident32_d = nc.dram_tensor("ident32", [128, 128], F32, kind="ExternalInput")
    wmap_d = nc.dram_tensor("wmap", [128, F_IN_PAD], BF16, kind="ExternalInput")
    bmap_d = nc.dram_tensor("bmap", [128, 1], F32, kind="ExternalInput")
    wl1_d = nc.dram_tensor("wl1", [128, D], BF16, kind="ExternalInput")
    wr1_d = nc.dram_tensor("wr1", [128, D], BF16, kind="ExternalInput")
    bl1_d = nc.dram_tensor("bl1", [128, 1], F32, kind="ExternalInput")
    wl2_d = nc.dram_tensor("wl2", [128, D2], BF16, kind="ExternalInput")
    wr2_d = nc.dram_tensor("wr2", [128, D2], BF16, kind="ExternalInput")
    bl2_d = nc.dram_tensor("bl2", [128, 2], F32, kind="ExternalInput")
    wl3_d = nc.dram_tensor("wl3", [128, 2 * NCLS_PAD], BF16, kind="ExternalInput")
    wr3_d = nc.dram_tensor("wr3", [128, 2 * NCLS_PAD], BF16, kind="ExternalInput")
    bl3_d = nc.dram_tensor("bl3", [NCLS_PAD, 1], F32, kind="ExternalInput")
    out_d = nc.dram_tensor("out", [NLOC, NCLS], F32, kind="ExternalOutput")

    # internal DRAM
    h0loc = nc.dram_tensor("h0loc", [NLOC, D], BF16)
    h1loc = nc.dram_tensor("h1loc", [NLOC, D], BF16)
    z3loc = nc.dram_tensor("z3loc", [NLOC, NCLS_PAD], BF16)
    h0full = nc.dram_tensor("h0full", [N, D], BF16, addr_space="Shared")
    h1full = nc.dram_tensor("h1full", [N, D], BF16, addr_space="Shared")
    z3full = nc.dram_tensor("z3full", [N, NCLS_PAD], BF16, addr_space="Shared")

    groups = [list(range(NCORES))]
    QHALF = NLOC // 2  # 3125 local rows per AG chunk

    rrq = [0]
    nxt = {0: 0, 1: 0}
    agq = []

    with tile.TileContext(nc) as tc:
        with (
            tc.tile_pool(name="const", bufs=1) as cp,
            tc.tile_pool(name="hres", bufs=1) as hp,
            tc.tile_pool(name="gatA", bufs=9) as gpA,
            tc.tile_pool(name="gatB", bufs=9) as gpB,
            tc.tile_pool(name="sone", bufs=3) as sp,
            tc.tile_pool(name="work", bufs=3) as wp,
            tc.tile_pool(name="xin", bufs=8) as xp,
            tc.tile_pool(name="pa", bufs=4, space="PSUM") as pa,
            tc.tile_pool(name="po", bufs=2, space="PSUM") as po,
            tc.tile_pool(name="pt", bufs=2, space="PSUM") as pt,
        ):
            # ---- load constants ----
            idx_sb = cp.tile([128, IX_COLS], I16)
            dl_sb = cp.tile([128, DL_COLS], F32)
            invdeg = cp.tile([128, NLOC_PAD], F32)
            iota = cp.tile([128, 128], F32)
            ident = cp.tile([128, 128], BF16)
            ident32 = cp.tile([128, 128], F32)
            wmap = cp.tile([128, F_IN_PAD], BF16)
            bmap = cp.tile([128, 1], F32)
            wl1 = cp.tile([128, D], BF16)
            wr1 = cp.tile([128, D], BF16)
            bl1 = cp.tile([128, 1], F32)
            wl2 = cp.tile([128, D2], BF16)
            wr2 = cp.tile([128, D2], BF16)
            bl2 = cp.tile([128, 2], F32)
            wl3 = cp.tile([128, 2 * NCLS_PAD], BF16)
            wr3 = cp.tile([128, 2 * NCLS_PAD], BF16)
            bl3 = cp.tile([NCLS_PAD, 1], F32)
            for sb_t, dr in [(idx_sb, idx_d), (dl_sb, dl_d), (invdeg, invdeg_d),
                             (iota, iota_d), (ident, ident_d),
                             (ident32, ident32_d), (wmap, wmap_d),
                             (bmap, bmap_d), (wl1, wl1_d), (wr1, wr1_d),
                             (bl1, bl1_d), (wl2, wl2_d), (wr2, wr2_d),
                             (bl2, bl2_d), (wl3, wl3_d), (wr3, wr3_d),
                             (bl3, bl3_d)]:
                nc.scalar.dma_start(out=sb_t[:], in_=dr[:])

            # persistent transposed activations (tags share slots over time)
            h0T = hp.tile([128, NLOC_PAD], BF16, tag="hA")
            h1T = hp.tile([128, NLOC_PAD], BF16, tag="hB")
            # SBUF accumulator for half-0 aggregation results
            aggT = hp.tile([128, NLOC_PAD], F32, tag="agg")

            # ---- stage 0: h0T = W_map^T @ xT + b_map ----
            # tiles are transposed + written per column window so the first
            # AllGather chunk fires early, overlapping the rest of stage 0
            def stage0_tile_out(t):
                ptr = pt.tile([128, 128], BF16, tag="pt")
                nc.tensor.transpose(ptr[:], h0T[:, _ts(t)], ident[:])
                nm = wp.tile([128, 128], BF16, tag="nm")
                nc.scalar.copy(nm[:], ptr[:])
                rows = min(128, NLOC - t * 128)
                nc.sync.dma_start(out=h0loc[t * 128: t * 128 + rows, :],
                                  in_=nm[0:rows, :])
                if t == 24 or t == NT - 1:
                    qn = 0 if t == 24 else 1
                    nc.gpsimd.collective_compute(
                        "AllGather", mybir.AluOpType.bypass,
                        replica_groups=groups,
                        ins=[h0loc[qn * QHALF:(qn + 1) * QHALF, :]],
                        outs=[h0full[qn * HALF:(qn + 1) * HALF, :]])

            NWIN = 1024
            n_wins = [(i * NWIN, min(NWIN, NLOC_PAD - i * NWIN))
                      for i in range((NLOC_PAD + NWIN - 1) // NWIN)]
            next_t = [0]
            for n0, nw in n_wins:
                slabs = [xp.tile([128, NWIN], BF16, tag="xs",
                                 name=f"xs{n0}_{k}") for k in range(4)]
                for k in range(4):
                    nc.sync.dma_start(out=slabs[k][:, 0:nw],
                                      in_=xT[_ts(k), n0:n0 + nw])
                for c0 in range(0, nw, 512):
                    cw = min(512, nw - c0)
                    ps = po.tile([128, 512], F32, tag="po")
                    for k in range(4):
                        nc.tensor.matmul(ps[:, 0:cw], wmap[:, _ts(k)],
                                         slabs[k][:, c0:c0 + cw],
                                         start=(k == 0), stop=(k == 3))
                    g0 = n0 + c0
                    nc.vector.tensor_scalar(out=h0T[:, g0:g0 + cw],
                                            in0=ps[:, 0:cw],
                                            scalar1=bmap[:, 0:1], scalar2=None,
                                            op0=mybir.AluOpType.add)
                while (next_t[0] + 1) * 128 <= n0 + nw and next_t[0] < NT:
                    stage0_tile_out(next_t[0])
                    next_t[0] += 1

            # ---- generic layer machinery ----
            # gather pieces: 8 chunks (1024 idxs) per dma_gather, cut across
            # tile boundaries within a half; matmuls consume tile-aligned
            # chunk slices out of the piece tiles
            def issue_piece(pieces, pool, src_full, h, p):
                n = min(1024, HCH[h] * 128 - p * 1024)
                q = rrq[0] % NQ
                rrq[0] += 1
                g = pool.tile([128, 8, 128], BF16, tag="g")
                io0 = (hch_off[h] * 8 + p * 64)
                nc.gpsimd.dma_gather(
                    g[:, 0:n // 128, :],
                    src_full[h * HALF:(h + 1) * HALF, :],
                    idx_sb[:, io0:io0 + n // 16],
                    n, n, D,
                    single_packet=True, queue_num=q)
                pieces[(h, p)] = g

            def agg_half(h, t, pieces, pool, src_full, pa_tile):
                """One tile's aggregation matmuls for one half."""
                s = h * NT + t
                nch_t = int(nch_flat[s])
                c0 = int(dl_off[s]) - hch_off[h]   # chunk offset within half
                last_piece = (c0 + nch_t - 1) // 8
                while nxt[h] <= last_piece:
                    issue_piece(pieces, pool, src_full, h, nxt[h])
                    nxt[h] += 1
                d0 = int(dl_off[s])
                sone = sp.tile([128, 12, 128], BF16, tag="S")
                io_b = iota[:].rearrange("p (o j) -> p o j", o=1) \
                    .broadcast_to([128, nch_t, 128])
                dl_b = dl_sb[:, d0:d0 + nch_t] \
                    .rearrange("p (c o) -> p c o", o=1) \
                    .broadcast_to([128, nch_t, 128])
                nc.vector.tensor_tensor(out=sone[:, 0:nch_t, :], in0=io_b,
                                        in1=dl_b, op=mybir.AluOpType.is_equal)
                for cc in range(nch_t):
                    gc = c0 + cc
                    g = pieces[(h, gc // 8)]
                    nc.tensor.matmul(pa_tile[:], g[:, gc % 8, :],
                                     sone[:, cc, :],
                                     start=(cc == 0), stop=(cc == nch_t - 1))

            def layer_loop(pool, src_full, tile_body, mean_dt=BF16):
                # software-pipelined halves: the half-1 visit of tile t runs
                # KPIPE tiles behind the half-0 visit, so tile bodies spread
                # over the whole layer while half-0 gathers (which only need
                # AllGather chunk 0) fill the window until chunk 1 lands.
                KPIPE = 12
                pieces = {}
                nxt[0] = nxt[1] = 0
                for i in range(NT + KPIPE):
                    if i < NT:
                        t = i
                        p = pa.tile([128, 128], F32, tag="pa")
                        agg_half(0, t, pieces, pool, src_full, p)
                        nc.scalar.copy(aggT[:, _ts(t)], p[:])
                    if i >= KPIPE:
                        t = i - KPIPE
                        p = pa.tile([128, 128], F32, tag="pa")
                        agg_half(1, t, pieces, pool, src_full, p)
                        m32 = wp.tile([128, 128], F32, tag="m32")
                        nc.vector.tensor_tensor(out=m32[:], in0=p[:],
                                                in1=aggT[:, _ts(t)],
                                                op=mybir.AluOpType.add)
                        mean = wp.tile([128, 128], mean_dt, tag="mean")
                        nc.vector.tensor_tensor(out=mean[:], in0=m32[:],
                                                in1=invdeg[:, _ts(t)],
                                                op=mybir.AluOpType.mult)
                        tile_body(t, mean)

            def ag_emit(loc, full, t):
                if t == 24 or t == NT - 1:
                    qn = 0 if t == 24 else 1
                    nc.gpsimd.collective_compute(
                        "AllGather", mybir.AluOpType.bypass,
                        replica_groups=groups,
                        ins=[loc[qn * QHALF:(qn + 1) * QHALF, :]],
                        outs=[full[qn * HALF:(qn + 1) * HALF, :]])

            # ---- layer 1 ----
            def l1_body(t, mean):
                rows = min(128, NLOC - t * 128)
                p1 = po.tile([128, 512], F32, tag="po")
                nc.tensor.matmul(p1[:, 0:128], wl1[:], mean[:],
                                 start=True, stop=False)
                nc.tensor.matmul(p1[:, 0:128], wr1[:], h0T[:, _ts(t)],
                                 start=False, stop=True)
                nc.scalar.activation(out=h1T[:, _ts(t)], in_=p1[:, 0:128],
                                     func=mybir.ActivationFunctionType.Relu,
                                     bias=bl1[:, 0:1], scale=1.0)
                ptr = pt.tile([128, 128], BF16, tag="pt")
                nc.tensor.transpose(ptr[:], h1T[:, _ts(t)], ident[:])
                nm = wp.tile([128, 128], BF16, tag="nm")
                nc.scalar.copy(nm[:], ptr[:])
                nc.sync.dma_start(out=h1loc[t * 128: t * 128 + rows, :],
                                  in_=nm[0:rows, :])
                ag_emit(h1loc, h1full, t)

            if build_stage >= 1:
                layer_loop(gpA, h0full, l1_body)

            # ---- layer 2 (+ z3 projection) ----
            h2T0 = hp.tile([128, NLOC_PAD], BF16, tag="hA")  # reuses h0T slot
            h2T1 = hp.tile([128, NLOC_PAD], BF16, tag="hC")

            def l2_body(t, mean):
                rows = min(128, NLOC - t * 128)
                for hh, (h2T_h, wcol) in enumerate(((h2T0, _ts(0)),
                                                    (h2T1, _ts(1)))):
                    p2 = po.tile([128, 512], F32, tag="po")
                    nc.tensor.matmul(p2[:, 0:128], wl2[:, wcol], mean[:],
                                     start=True, stop=False)
                    nc.tensor.matmul(p2[:, 0:128], wr2[:, wcol], h1T[:, _ts(t)],
                                     start=False, stop=True)
                    nc.scalar.activation(
                        out=h2T_h[:, _ts(t)], in_=p2[:, 0:128],
                        func=mybir.ActivationFunctionType.Relu,
                        bias=bl2[:, hh:hh + 1], scale=1.0)
                # z3 = h2 @ Wl3 (transposed: z3T = Wl3^T @ h2T)
                pz = po.tile([128, 512], F32, tag="po")
                nc.tensor.matmul(pz[0:NCLS_PAD, 0:128], wl3[:, 0:NCLS_PAD],
                                 h2T0[:, _ts(t)], start=True, stop=False)
                nc.tensor.matmul(pz[0:NCLS_PAD, 0:128],
                                 wl3[:, NCLS_PAD:2 * NCLS_PAD],
                                 h2T1[:, _ts(t)], start=False, stop=True)
                zt = wp.tile([NCLS_PAD, 128], BF16, tag="zt")
                nc.scalar.copy(zt[:], pz[0:NCLS_PAD, 0:128])
                ptz = pt.tile([128, 128], BF16, tag="pt")
                nc.tensor.transpose(ptz[:, 0:NCLS_PAD], zt[:],
                                    ident[0:NCLS_PAD, 0:NCLS_PAD])
                nmz = wp.tile([128, 128], BF16, tag="nm")
                nc.scalar.copy(nmz[:, 0:NCLS_PAD], ptz[:, 0:NCLS_PAD])
                nc.sync.dma_start(out=z3loc[t * 128: t * 128 + rows, :],
                                  in_=nmz[0:rows, 0:NCLS_PAD])
                ag_emit(z3loc, z3full, t)

            if build_stage >= 2:
                layer_loop(gpB, h1full, l2_body)

            # ---- layer 3 + log_softmax ----
            def l3_body(t, mean):
                rows = min(128, NLOC - t * 128)
                p3 = po.tile([128, 512], F32, tag="po")
                nc.tensor.matmul(p3[0:NCLS_PAD, 0:128], wr3[:, 0:NCLS_PAD],
                                 h2T0[:, _ts(t)], start=True, stop=False)
                nc.tensor.matmul(p3[0:NCLS_PAD, 0:128],
                                 wr3[:, NCLS_PAD:2 * NCLS_PAD],
                                 h2T1[:, _ts(t)], start=False, stop=True)
                # mean (already inv-deg scaled) + wr3 term + bias
                W64 = 64
                comb = wp.tile([W64, 128], F32, tag="comb")
                nc.vector.tensor_tensor(out=comb[:], in0=mean[0:W64, :],
                                        in1=p3[0:W64, 0:128],
                                        op=mybir.AluOpType.add)
                comb2 = wp.tile([W64, 128], F32, tag="comb2")
                nc.scalar.activation(out=comb2[:], in_=comb[:],
                                     func=mybir.ActivationFunctionType.Identity,
                                     bias=bl3[0:W64, 0:1], scale=1.0)
                ptf = pt.tile([128, 128], F32, tag="pt")
                nc.tensor.transpose(ptf[:, 0:W64], comb2[:],
                                    ident32[0:W64, 0:W64])
                # log_softmax over the 40 valid class columns
                xm = wp.tile([128, 1], F32, tag="xm")
                nc.vector.tensor_reduce(out=xm[:], in_=ptf[:, 0:NCLS],
                                        axis=mybir.AxisListType.X,
                                        op=mybir.AluOpType.max, negate=True)
                tt = wp.tile([128, NCLS], F32, tag="tt")
                nc.scalar.activation(out=tt[:], in_=ptf[:, 0:NCLS],
                                     func=mybir.ActivationFunctionType.Identity,
                                     bias=xm[:, 0:1], scale=1.0)
                ex = wp.tile([128, NCLS], F32, tag="ex")
                ssum = wp.tile([128, 1], F32, tag="ssum")
                nc.scalar.activation(out=ex[:], in_=tt[:],
                                     func=mybir.ActivationFunctionType.Exp,
                                     accum_out=ssum[:])
                lse = wp.tile([128, 1], F32, tag="lse")
                nc.scalar.activation(out=lse[:], in_=ssum[:],
                                     func=mybir.ActivationFunctionType.Ln)
                lsn = wp.tile([128, 1], F32, tag="lsn")
                nc.scalar.mul(lsn[:], lse[:], -1.0)
                fin = wp.tile([128, NCLS], F32, tag="fin")
                nc.scalar.activation(out=fin[:], in_=tt[:],
                                     func=mybir.ActivationFunctionType.Identity,
                                     bias=lsn[:, 0:1], scale=1.0)
                nc.sync.dma_start(out=out_d[t * 128: t * 128 + rows, :],
                                  in_=fin[0:rows, :])

            if build_stage >= 3:
                layer_loop(gpA, z3full, l3_body, mean_dt=F32)

    nc.compile()
    return nc


# ---------------- host side ----------------

def _pack_idx_segment(vals: np.ndarray) -> np.ndarray:
    """[L] int16 -> [128, L//16]: slot i -> [i % 16, i // 16], x8 replicated."""
    L = vals.shape[0]
    arr = vals.reshape(L // 16, 16).T  # [16, L//16]
    return np.tile(arr, (8, 1))


def prepare_inputs(x, edge_index, W_map, b_map, Wl1, bl1, Wr1, Wl2, bl2, Wr2,
                   Wl3, bl3, Wr3):
    src = np.asarray(edge_index[0], dtype=np.int64)
    dst = np.asarray(edge_index[1], dtype=np.int64)

    core = dst // NLOC
    local = dst - core * NLOC
    t_loc = local >> 7
    dloc = local & 127
    # AG-chunk table layout: chunk q holds all cores' local rows
    # [q*3125, (q+1)*3125): table row = c_src*3125 + (r - q*3125)
    c_src = src // NLOC
    r_src = src - c_src * NLOC
    half = (r_src >= NLOC // 2).astype(np.int64)
    idx16 = (c_src * (NLOC // 2) + (r_src - half * (NLOC // 2))).astype(np.int16)

    # fine group (core, half, tile)
    fine = (core * 2 + half) * NT + t_loc
    NFINE = NCORES * 2 * NT
    counts = np.bincount(fine, minlength=NFINE).reshape(NCORES, 2 * NT)
    nch_flat = np.maximum(1, np.ceil(counts.max(axis=0) / 128).astype(int))
    nch = nch_flat.reshape(2, NT)
    seg_slots = nch_flat * 128
    seg_off = np.concatenate([[0], np.cumsum(seg_slots)])
    SLTOT = int(seg_off[-1])

    order = np.argsort(fine, kind="stable")
    fine_s = fine[order]
    cnt_all = np.bincount(fine, minlength=NFINE)
    offs = np.concatenate([[0], np.cumsum(cnt_all)])
    pos = np.arange(E) - np.repeat(offs[:-1], cnt_all)
    # slot within the owning core's table
    seg_of_edge = fine_s % (2 * NT)
    slot = seg_off[seg_of_edge] + pos
    core_s = fine_s // (2 * NT)

    # pad slots gather row 0 (benign; dl=999 keeps them out of the one-hot)
    big_idx = np.zeros((NCORES, SLTOT), dtype=np.int16)
    big_dl = np.full((NCORES, SLTOT), 999.0, dtype=np.float32)
    big_idx[core_s, slot] = idx16[order]
    big_dl[core_s, slot] = dloc[order].astype(np.float32)

    # degrees
    cnt = np.bincount(dst, minlength=N).astype(np.float32)
    inv = 1.0 / np.maximum(cnt, 1.0)

    # weights (shared)
    BF = ml_dtypes.bfloat16
    Wmap_pad = np.zeros((F_IN_PAD, 128), np.float32)
    Wmap_pad[0:F_IN] = W_map
    wmap_kt = np.concatenate([Wmap_pad[_ts(k)] for k in range(4)], axis=1)
    Wl3_pad = np.zeros((D2, NCLS_PAD), np.float32)
    Wl3_pad[:, 0:NCLS] = Wl3
    wl3_kt = np.concatenate([Wl3_pad[_ts(k)] for k in range(2)], axis=1)
    Wr3_pad = np.zeros((D2, NCLS_PAD), np.float32)
    Wr3_pad[:, 0:NCLS] = Wr3
    wr3_kt = np.concatenate([Wr3_pad[_ts(k)] for k in range(2)], axis=1)
    bl3_pad = np.zeros((NCLS_PAD, 1), np.float32)
    bl3_pad[0:NCLS, 0] = bl3

    shared = {
        "iota": np.ascontiguousarray(
            np.tile(np.arange(128, dtype=np.float32), (128, 1))),
        "ident": np.eye(128, dtype=np.float32).astype(BF),
        "ident32": np.eye(128, dtype=np.float32),
        "wmap": np.ascontiguousarray(wmap_kt).astype(BF),
        "bmap": np.ascontiguousarray(b_map.reshape(128, 1)),
        "wl1": np.ascontiguousarray(Wl1).astype(BF),
        "wr1": np.ascontiguousarray(Wr1).astype(BF),
        "bl1": np.ascontiguousarray(bl1.reshape(128, 1)),
        "wl2": np.ascontiguousarray(Wl2).astype(BF),
        "wr2": np.ascontiguousarray(Wr2).astype(BF),
        "bl2": np.ascontiguousarray(bl2.reshape(2, 128).T),
        "wl3": np.ascontiguousarray(wl3_kt).astype(BF),
        "wr3": np.ascontiguousarray(wr3_kt).astype(BF),
        "bl3": bl3_pad,
    }

    in_maps = []
    for c in range(NCORES):
        xT_pad = np.zeros((F_IN_PAD, NLOC_PAD), np.float32)
        xT_pad[0:F_IN, 0:NLOC] = x[c * NLOC:(c + 1) * NLOC].T
        xT_pad = xT_pad.astype(ml_dtypes.bfloat16)

        # idx packed per HALF (gather pieces slice 1024-idx windows out of
        # the half's packed block); dl cols = global chunk index
        half_bound = int(seg_off[NT])  # slots in half 0
        idx_arr = np.ascontiguousarray(np.concatenate(
            [_pack_idx_segment(big_idx[c, 0:half_bound]),
             _pack_idx_segment(big_idx[c, half_bound:])], axis=1))
        dl_arr = np.ascontiguousarray(big_dl[c].reshape(-1, 128).T)

        inv_pad = np.ones(NLOC_PAD, np.float32)
        inv_pad[0:NLOC] = inv[c * NLOC:(c + 1) * NLOC]
        invdeg_arr = np.ascontiguousarray(
            np.broadcast_to(inv_pad, (128, NLOC_PAD)))

        m = {
            "xT": xT_pad,
            "idx": idx_arr,
            "dl": dl_arr,
            "invdeg": invdeg_arr,
        }
        m.update(shared)
        in_maps.append(m)
    return in_maps, nch


_prog_cache = {}


def kernel(**inputs) -> np.ndarray:
    args = {k: np.asarray(v) for k, v in inputs.items()}
    in_maps, nch = prepare_inputs(
        args["x"], args["edge_index"], args["W_map"], args["b_map"],
        args["Wl1"], args["bl1"], args["Wr1"], args["Wl2"], args["bl2"],
        args["Wr2"], args["Wl3"], args["bl3"], args["Wr3"])

    key = tuple(nch.reshape(-1).tolist())
    if key not in _prog_cache:
        _prog_cache[key] = build_program(nch)
    nc = _prog_cache[key]

    trace = os.environ.get("KERNEL_TRACE", "0") == "1"
    kw = {}
    if trace:
        import concourse.bass_utils as bu
        bu.upload_artifacts = lambda t: ""
        kw = dict(trace=True, tmpdir=os.environ.get(
            "KERNEL_TRACE_DIR", "/tmp/kernel_trace"))
    res = run_bass_kernel_spmd(nc, in_maps, list(range(NCORES)), **kw)
    if trace and res.exec_time_ns is not None:
        print(f"HW exec time: {res.exec_time_ns} ns")

    out = np.concatenate([res.results[c]["out"] for c in range(NCORES)], axis=0)
    return out.astype(np.float32)
